# revision 14
# baseline (speedup 1.0000x reference)
"""EuclidConv + training-mode BatchNorm on 8 Trainium2 NeuronCores.

Math (reference): out = BN(2*conv(x,w) + conv(x^2, ones3x3) + ||w_f||^2),
BN over global batch stats. The per-filter ||w||^2 term is channel-constant,
so BN's mean subtraction cancels it exactly -> never computed.

Sharding: OUTPUT-CHANNEL sharded (32 of 256 channels per core, all 32
images). This keeps the BN statistics entirely core-local (cross-core
collectives cost 40-50us of measured span in this environment: ~10-16us
ncfw mesh floor plus 25-40us core launch skew that the graded core-0 span
absorbs). Images pack 4-at-a-time into the 128x128 PE array via
tile_position col-tiling: psum partition p = 32*j + c for image-slot j,
channel c. fp16 operands (11-bit mantissa ~ f32r precision at half the
bytes, full PE rate).

Per 4-image block b (image slots j=0..3):
  u_j = x_j^2                                      (vector, fp16)
  r4 psum[32j..32j+32] = ones32.T @ u_j            (channel sums, replicated
                                                    over the 32 channel rows)
  rc = r4 - 128*validmap                           (vector, fp16, centered
                                                    for precision)
  box filter: vv = 3-tap vertical adds (vector), t1f = 3-tap horizontal
  adds (gpsimd; keeps vector free for the next block's square)
  conv psum accumulation group (one [128,1024] 2-bank tile, yt halves):
    identity.T @ t1f_view   (start=True: adds t1, clears psum)
    sum_k (2w)_k.T @ x_view (9 offsets x 4 col-tiles, fp16)
    ones1.T @ countmap_view (stop=True: re-adds 128*count, undoing centering)
  drain: ACT copy psum->s_sb with accum S; ACT square with accum Q (scalar)
Stats tail (all on [128,*], channel stats REPLICATED to every image slot by
one mask matmul; no 32->128 broadcast chain): reduce S,Q cols; bigmask
(q==p mod 32) matmul -> gstat128 [128,2]; mean/var/rsqrt/A/B elementwise;
normalize out = s*A+B split across vector/scalar/gpsimd; 8 full-image
output DMAs (3136B/partition descriptors) issued on idle engines.

Host-side prep is layout/sharding only: pad+transpose+cast of x, weight
transpose/scale, constant masks.
"""
import json

import numpy as np

import concourse.bass as bass
import concourse.mybir as mybir
import concourse.tile as tile
from concourse.ap import AP
from concourse.bass_utils import run_bass_kernel_spmd
from concourse.vector_clock import ScopedClock, VectorClock

F16 = mybir.dt.float16
F32 = mybir.dt.float32

N_CORES = 8
NIMG = 32
NBLK = 8
HP = 30
NPIX = HP * HP
NV = 28 * 28
YT_ROWS = 14
YT = YT_ROWS * 28
NHW = NIMG * NV
EPS = 1e-5

_split_ctr = [0]


def _split_waits_json(bir: bytes, max_waits: int = 1) -> bytes:
    """This container's walrus rejects instructions with >1 sync wait.
    Hoist excess waits onto EventSemaphore instructions inserted before the
    offender on the same engine stream."""
    m = json.loads(bir)
    for f in m["functions"]:
        for bb in f["blocks"]:
            newinsts = []
            for ins in bb["instructions"]:
                si = ins.get("sync_info")
                if si:
                    waits = si.get("on_wait") or []
                    if len(waits) > max_waits:
                        extra, keep = waits[:-max_waits], waits[-max_waits:]
                        for w_ in extra:
                            _split_ctr[0] += 1
                            newinsts.append(
                                {
                                    "debug": ins.get("debug", 0),
                                    "engine": ins["engine"],
                                    "ins": [],
                                    "outs": [],
                                    "name": f"antsplitw-{_split_ctr[0]}",
                                    "opcode": "EventSemaphore",
                                    "sync_info": {"on_update": [], "on_wait": [w_]},
                                }
                            )
                        si["on_wait"] = keep
                newinsts.append(ins)
            bb["instructions"] = newinsts
    return json.dumps(m).encode()


class _PatchedBass(bass.Bass):
    def to_json_bytes(self):
        return _split_waits_json(super().to_json_bytes())


class _SplitDrainTileContext(tile.TileContext):
    """Split the tile-exit drain's waits into single-wait drains (same
    walrus limitation as above)."""

    def _drain_and_barrier(self, tick_clock, wait_clock):
        g = tick_clock.global_clock
        n = len(g)
        for i in range(n):
            if g[i] > 0:
                vec = [0] * n
                vec[i] = g[i]
                d = self.nc.sync.drain()
                wait_clock.add_sem_waits(d.ins, ScopedClock({None: VectorClock(vec)}))
        self.nc.sync.drain()
        self.nc.all_engine_barrier()
        assert self.sems is not None
        popped = self.nc._tile_sem_poison_stack.pop()
        assert popped is self._sem_poison
        self.nc.clear_and_free_semaphores(list(self.sems.allocated().values()))
        self.nc.all_engine_barrier()


def _build_nc():
    nc = _PatchedBass()
    xh = nc.dram_tensor("xh", [128, NIMG * NPIX], F16, kind="ExternalInput")
    wt = nc.dram_tensor("wt", [128, 9 * 32], F16, kind="ExternalInput")
    # packed fp16 constants: [ones32 | comp 904 | cmap 840 pad to 1 row.. ]
    c16a = nc.dram_tensor("c16a", [128, 32 + 128], F16, kind="ExternalInput")
    cst32 = nc.dram_tensor("cst32", [128, 136], F32, kind="ExternalInput")
    y = nc.dram_tensor("y", [NIMG, 32, 28, 28], F16, kind="ExternalOutput")

    with _SplitDrainTileContext(nc) as tc:
        with (
            tc.tile_pool(name="const", bufs=1) as cpool,
            tc.tile_pool(name="xpool", bufs=1) as xpool,
            tc.tile_pool(name="upool", bufs=1) as upool,
            tc.tile_pool(name="rpool", bufs=2) as rpool,
            tc.tile_pool(name="spool", bufs=1) as spool,
            tc.tile_pool(name="opool", bufs=3) as opool,
            tc.tile_pool(name="psc", bufs=2, space="PSUM") as psc,
            tc.tile_pool(name="psr", bufs=2, space="PSUM") as psr,
        ):
            wtile = cpool.tile([128, 9 * 32], F16, name="wtile")
            nc.scalar.dma_start(wtile[:], wt[:])
            c16 = cpool.tile([128, 32 + 128], F16, name="c16")
            nc.scalar.dma_start(c16[:], c16a[:])
            c32 = cpool.tile([128, 136], F32, name="c32")
            nc.scalar.dma_start(c32[:], cst32[:])
            ones32 = c16[0:128, 0:32]
            idt = c16[:, 32:160]
            bigmask = c32[:, 0:128]
            gamma128 = c32[:, 128:129]
            beta128 = c32[:, 129:130]
            epsb = c32[:, 130:131]
            biasC = c32[:, 131:132]

            s_sb = spool.tile([128, 16 * YT], F16, name="s_sb")
            sums = spool.tile([128, 8], F32, name="sums")
            sumsq = spool.tile([128, 8], F32, name="sumsq")

            xall = xpool.tile([128, NIMG * NPIX], F16, name="xall")
            nc.sync.dma_start(xall[:, 0 : 2 * NPIX], xh[:, 0 : 2 * NPIX])
            nc.sync.dma_start(
                xall[:, 2 * NPIX : 4 * NPIX], xh[:, 2 * NPIX : 4 * NPIX]
            )
            for b in range(1, NBLK):
                sl = slice(b * 4 * NPIX, (b + 1) * 4 * NPIX)
                nc.sync.dma_start(xall[:, sl], xh[:, sl])

            # squares on scalar ACT, issued with +2 lookahead so the in-order
            # scalar queue never makes a drain wait on a square
            uts = [
                upool.tile([128, 4 * NPIX], F16, name=f"ut{b}", tag=f"ut{b % 3}")
                for b in range(NBLK)
            ]

            def _issue_square_v(b):
                lo = b * 4 * NPIX
                nc.vector.tensor_mul(
                    uts[b][:, 0 : 2 * NPIX],
                    xall[:, lo : lo + 2 * NPIX],
                    xall[:, lo : lo + 2 * NPIX],
                )

            def _issue_square_s(b):
                lo = b * 4 * NPIX
                nc.scalar.activation(
                    uts[b][:, 2 * NPIX : 4 * NPIX],
                    xall[:, lo + 2 * NPIX : lo + 4 * NPIX],
                    mybir.ActivationFunctionType.Square,
                )

            def _issue_square(b):
                _issue_square_v(b)
                _issue_square_s(b)

            _issue_square(0)
            _issue_square_s(1)
            _issue_square_s(2)

            def _issue_r4(b):
                ut = uts[b]
                r4 = psr.tile([128, 904], F32, name=f"r4_{b}", tag="r4")
                for j in (2, 3, 0, 1):
                    for lo, hi in ((0, 512), (512, 900)):
                        nc.tensor.matmul(
                            r4[32 * j : 32 * j + 32, lo:hi],
                            ones32,
                            ut[:, j * NPIX + lo : j * NPIX + hi],
                            start=True,
                            stop=True,
                            tile_position=(0, 32 * j),
                            skip_group_check=True,
                        )
                return r4

            r4s = {0: _issue_r4(0)}

            for b in range(NBLK):
                r4 = r4s.pop(b)
                rc16 = rpool.tile([128, 904], F16, name=f"rc16_{b}", tag="rc16")
                nc.vector.tensor_scalar(
                    rc16[:, 0:900],
                    r4[:, 0:900],
                    96.0,
                    None,
                    op0=mybir.AluOpType.subtract,
                )
                vv = rpool.tile([128, 840], F16, name=f"vv{b}", tag="vv")
                nc.vector.tensor_add(vv[:], rc16[:, 0:840], rc16[:, 30:870])
                nc.vector.tensor_add(vv[:], vv[:], rc16[:, 60:900])
                t1f = rpool.tile([128, 840], F16, name=f"t1f{b}", tag="t1f")
                nc.vector.tensor_add(t1f[:, 0:838], vv[:, 0:838], vv[:, 1:839])
                nc.vector.tensor_add(t1f[:, 0:838], t1f[:, 0:838], vv[:, 2:840])
                if b == 0:
                    _issue_square_v(1)
                if b + 2 < NBLK:
                    _issue_square_v(b + 2)
                if b + 3 < NBLK:
                    _issue_square_s(b + 3)

                x3 = xall[:].rearrange("p (n a b) -> p n a b", a=HP, b=HP)
                ps = psc.tile([128, 1024], F32, name=f"ps{b}", tag="ps")
                t13 = t1f[:].rearrange("p (a c) -> p a c", c=HP)
                for yt in range(2):
                    y0 = yt * YT_ROWS
                    nc.tensor.matmul(
                        ps[:, 512 * yt : 512 * yt + YT],
                        idt,
                        t13[:, y0 : y0 + YT_ROWS, 0:28],
                        start=True,
                        stop=False,
                        skip_group_check=True,
                    )
                for yt in range(2):
                    y0 = yt * YT_ROWS
                    for k in range(9):
                        dy, dx = divmod(k, 3)
                        for j in range(4):
                            nc.tensor.matmul(
                                ps[32 * j : 32 * j + 32, 512 * yt : 512 * yt + YT],
                                wtile[:, k * 32 : (k + 1) * 32],
                                x3[:, b * 4 + j, y0 + dy : y0 + dy + YT_ROWS, dx : dx + 28],
                                start=False,
                                stop=(k == 8),
                                tile_position=(0, 32 * j),
                                skip_group_check=True,
                            )
                    if yt == 0 and b + 1 < NBLK:
                        r4s[b + 1] = _issue_r4(b + 1)
                blk = b * 2 * YT
                psv = AP(ps.tensor, ps.offset, [[1024, 128], [512, 2], [1, YT]])
                nc.scalar.activation(
                    s_sb[:, blk : blk + 2 * YT],
                    psv,
                    mybir.ActivationFunctionType.Copy,
                    bias=-288.0,
                    accum_out=sums[:, b : b + 1],
                )
                sq_scr = opool.tile([128, 2 * YT], F32, name="sq_scr", tag="sq")
                nc.scalar.activation(
                    sq_scr[:],
                    s_sb[:, blk : blk + 2 * YT],
                    mybir.ActivationFunctionType.Square,
                    accum_out=sumsq[:, b : b + 1],
                )

            # ---- stats tail: everything on [128, *] (channel stats
            # replicated across the 4 image slots by the bigmask matmul) ----
            sq2 = spool.tile([128, 2], F32, name="sq2")
            nc.vector.tensor_reduce(
                out=sq2[:, 0:1], in_=sums[:], op=mybir.AluOpType.add,
                axis=mybir.AxisListType.X,
            )
            nc.vector.tensor_reduce(
                out=sq2[:, 1:2], in_=sumsq[:], op=mybir.AluOpType.add,
                axis=mybir.AxisListType.X,
            )
            gstat = psr.tile([128, 2], F32, name="gstat", tag="r4")
            nc.tensor.matmul(gstat[:], bigmask, sq2[:], start=True, stop=True)
            ab = spool.tile([128, 8], F32, name="ab")
            nc.vector.tensor_copy(ab[:, 0:2], gstat[:])
            mean = ab[:, 0:1]
            qn = ab[:, 1:2]
            var = ab[:, 2:3]
            nc.vector.scalar_tensor_tensor(
                var, mean, 1.0, mean, op0=mybir.AluOpType.mult, op1=mybir.AluOpType.mult
            )
            nc.vector.tensor_sub(var, qn, var)
            abv = spool.tile([128, 2], F32, name="abv")
            A = abv[:, 0:1]
            B = abv[:, 1:2]
            sd = ab[:, 3:4]
            nc.scalar.activation(
                sd, var, mybir.ActivationFunctionType.Sqrt, bias=epsb
            )
            nc.vector.reciprocal(A, sd)
            nc.vector.tensor_mul(A, A, gamma128)
            nc.vector.scalar_tensor_tensor(
                B, mean, 1.0, A, op0=mybir.AluOpType.mult, op1=mybir.AluOpType.mult
            )
            nc.vector.tensor_sub(B, beta128, B)

            for b in range(NBLK):
                blk = b * 2 * YT
                o = opool.tile([128, 2 * YT], F16, name=f"o{b}", tag=f"o{b % 3}")
                if b % 3 == 0:
                    nc.vector.tensor_scalar(
                        o[:],
                        s_sb[:, blk : blk + 2 * YT],
                        A,
                        B,
                        op0=mybir.AluOpType.mult,
                        op1=mybir.AluOpType.add,
                    )
                elif b % 3 == 1:
                    nc.scalar.activation(
                        o[:],
                        s_sb[:, blk : blk + 2 * YT],
                        mybir.ActivationFunctionType.Identity,
                        bias=B,
                        scale=A,
                    )
                else:
                    nc.gpsimd.tensor_scalar(
                        o[:],
                        s_sb[:, blk : blk + 2 * YT],
                        A,
                        B,
                        op0=mybir.AluOpType.mult,
                        op1=mybir.AluOpType.add,
                    )
                dstap = AP(
                    y.ap().tensor,
                    b * 4 * 32 * NV,
                    [[32 * NV, 4], [NV, 32], [1, 2 * YT]],
                )
                eng = (nc.sync, nc.scalar, nc.gpsimd)[b % 3]
                eng.dma_start(dstap, o[:])
    return nc


def _prep_inputs(x, w, gamma, beta):
    x = np.asarray(x, np.float32)
    w = np.asarray(w, np.float32)
    gamma = np.asarray(gamma, np.float32)
    beta = np.asarray(beta, np.float32)

    xp = np.zeros((NIMG, 128, HP, HP), np.float32)
    xp[:, :, 1:29, 1:29] = x
    xh = np.ascontiguousarray(xp.transpose(1, 0, 2, 3)).reshape(128, NIMG * NPIX)
    xh = xh.astype(np.float16)

    c16a = np.zeros((128, 32 + 128), np.float16)
    c16a[:, 0:32] = 1.0
    c16a[:, 32:160] = np.eye(128, dtype=np.float16)

    bigmask = np.zeros((128, 128), np.float32)
    for c in range(128):
        bigmask[c, c % 32 :: 32] = 1.0
    # gstat128[p,:] = sum_q bigmask[q,p] * sq2[q,:]; want channel (p%32) sums
    # -> bigmask[q,p] = 1 iff q%32 == p%32
    bm = np.zeros((128, 128), np.float32)
    qq = np.arange(128)
    for p in range(128):
        bm[:, p] = (qq % 32 == p % 32).astype(np.float32) / NHW

    maps = []
    for core in range(N_CORES):
        wtc = (2.0 * w[core * 32 : (core + 1) * 32]).reshape(32, 128, 9)
        wtc = np.ascontiguousarray(wtc.transpose(1, 2, 0)).reshape(128, 9 * 32)
        cst32 = np.zeros((128, 136), np.float32)
        cst32[:, 0:128] = bm
        g4 = np.tile(gamma[core * 32 : (core + 1) * 32], 4)
        b4 = np.tile(beta[core * 32 : (core + 1) * 32], 4)
        cst32[:, 128] = g4
        cst32[:, 129] = b4
        cst32[:, 130] = EPS
        cst32[:, 131] = -288.0
        maps.append(
            {
                "xh": xh,
                "wt": wtc.astype(np.float16),
                "c16a": c16a,
                "cst32": cst32,
            }
        )
    return maps


_NC_CACHE = []


def kernel(x, w, gamma, beta):
    if not _NC_CACHE:
        _NC_CACHE.append(_build_nc())
    nc = _NC_CACHE[0]
    maps = _prep_inputs(x, w, gamma, beta)
    res = run_bass_kernel_spmd(nc, maps, core_ids=list(range(N_CORES)))
    out = np.concatenate([r["y"] for r in res.results], axis=1)
    return np.ascontiguousarray(out.astype(np.float32))


# revision 15
# speedup vs baseline: 1.2263x; 1.2263x over previous
"""EuclidConv + training-mode BatchNorm on 8 Trainium2 NeuronCores.

Math (reference): out = BN(2*conv(x,w) + conv(x^2, ones3x3) + ||w_f||^2),
BN over global batch stats. The per-filter ||w||^2 term is channel-constant,
so BN's mean subtraction cancels it exactly -> never computed.

Sharding: OUTPUT-CHANNEL sharded (32 of 256 channels per core, all 32
images). This keeps the BN statistics entirely core-local (cross-core
collectives cost 40-50us of measured span in this environment: ~10-16us
ncfw mesh floor plus 25-40us core launch skew that the graded core-0 span
absorbs). Images pack 4-at-a-time into the 128x128 PE array via
tile_position col-tiling: psum partition p = 32*j + c for image-slot j,
channel c. fp16 operands (11-bit mantissa ~ f32r precision at half the
bytes, full PE rate).

Per 4-image block b (image slots j=0..3):
  u_j = x_j^2                                      (vector, fp16)
  r4 psum[32j..32j+32] = ones32.T @ u_j            (channel sums, replicated
                                                    over the 32 channel rows)
  rc = r4 - 128*validmap                           (vector, fp16, centered
                                                    for precision)
  box filter: vv = 3-tap vertical adds (vector), t1f = 3-tap horizontal
  adds (gpsimd; keeps vector free for the next block's square)
  conv psum accumulation group (one [128,1024] 2-bank tile, yt halves):
    identity.T @ t1f_view   (start=True: adds t1, clears psum)
    sum_k (2w)_k.T @ x_view (9 offsets x 4 col-tiles, fp16)
    ones1.T @ countmap_view (stop=True: re-adds 128*count, undoing centering)
  drain: ACT copy psum->s_sb with accum S; ACT square with accum Q (scalar)
Stats tail (all on [128,*], channel stats REPLICATED to every image slot by
one mask matmul; no 32->128 broadcast chain): reduce S,Q cols; bigmask
(q==p mod 32) matmul -> gstat128 [128,2]; mean/var/rsqrt/A/B elementwise;
normalize out = s*A+B split across vector/scalar/gpsimd; 8 full-image
output DMAs (3136B/partition descriptors) issued on idle engines.

Host-side prep is layout/sharding only: pad+transpose+cast of x, weight
transpose/scale, constant masks.
"""
import json

import numpy as np

import concourse.bass as bass
import concourse.mybir as mybir
import concourse.tile as tile
from concourse.ap import AP
from concourse.bass_utils import run_bass_kernel_spmd
from concourse.vector_clock import ScopedClock, VectorClock

F16 = mybir.dt.float16
F32 = mybir.dt.float32

N_CORES = 8
NIMG = 32
NBLK = 8
HP = 30
NPIX = HP * HP
NV = 28 * 28
YT_ROWS = 14
YT = YT_ROWS * 28
NHW = NIMG * NV
EPS = 1e-5

_split_ctr = [0]


def _split_waits_json(bir: bytes, max_waits: int = 1) -> bytes:
    """This container's walrus rejects instructions with >1 sync wait.
    Hoist excess waits onto EventSemaphore instructions inserted before the
    offender on the same engine stream."""
    m = json.loads(bir)
    for f in m["functions"]:
        for bb in f["blocks"]:
            newinsts = []
            for ins in bb["instructions"]:
                si = ins.get("sync_info")
                if si:
                    waits = si.get("on_wait") or []
                    if len(waits) > max_waits:
                        extra, keep = waits[:-max_waits], waits[-max_waits:]
                        for w_ in extra:
                            _split_ctr[0] += 1
                            newinsts.append(
                                {
                                    "debug": ins.get("debug", 0),
                                    "engine": ins["engine"],
                                    "ins": [],
                                    "outs": [],
                                    "name": f"antsplitw-{_split_ctr[0]}",
                                    "opcode": "EventSemaphore",
                                    "sync_info": {"on_update": [], "on_wait": [w_]},
                                }
                            )
                        si["on_wait"] = keep
                newinsts.append(ins)
            bb["instructions"] = newinsts
    return json.dumps(m).encode()


class _PatchedBass(bass.Bass):
    def to_json_bytes(self):
        return _split_waits_json(super().to_json_bytes())


class _SplitDrainTileContext(tile.TileContext):
    """Split the tile-exit drain's waits into single-wait drains (same
    walrus limitation as above)."""

    def _drain_and_barrier(self, tick_clock, wait_clock):
        g = tick_clock.global_clock
        n = len(g)
        for i in range(n):
            if g[i] > 0:
                vec = [0] * n
                vec[i] = g[i]
                d = self.nc.sync.drain()
                wait_clock.add_sem_waits(d.ins, ScopedClock({None: VectorClock(vec)}))
        self.nc.sync.drain()
        self.nc.all_engine_barrier()
        assert self.sems is not None
        popped = self.nc._tile_sem_poison_stack.pop()
        assert popped is self._sem_poison
        self.nc.clear_and_free_semaphores(list(self.sems.allocated().values()))
        self.nc.all_engine_barrier()


def _build_nc():
    nc = _PatchedBass()
    xh = nc.dram_tensor("xh", [128, NIMG * NPIX], F16, kind="ExternalInput")
    wt = nc.dram_tensor("wt", [128, 9 * 32], F16, kind="ExternalInput")
    # packed fp16 constants: [ones32 | comp 904 | cmap 840 pad to 1 row.. ]
    c16a = nc.dram_tensor("c16a", [128, 32 + 128], F16, kind="ExternalInput")
    cst32 = nc.dram_tensor("cst32", [128, 136], F32, kind="ExternalInput")
    y = nc.dram_tensor("y", [NIMG, 32, 28, 28], F16, kind="ExternalOutput")

    with _SplitDrainTileContext(nc) as tc:
        with (
            tc.tile_pool(name="const", bufs=1) as cpool,
            tc.tile_pool(name="xpool", bufs=1) as xpool,
            tc.tile_pool(name="upool", bufs=1) as upool,
            tc.tile_pool(name="rpool", bufs=2) as rpool,
            tc.tile_pool(name="spool", bufs=1) as spool,
            tc.tile_pool(name="opool", bufs=3) as opool,
            tc.tile_pool(name="psc", bufs=2, space="PSUM") as psc,
            tc.tile_pool(name="psr", bufs=2, space="PSUM") as psr,
        ):
            wtile = cpool.tile([128, 9 * 32], F16, name="wtile")
            nc.scalar.dma_start(wtile[:], wt[:])
            c16 = cpool.tile([128, 32 + 128], F16, name="c16")
            nc.scalar.dma_start(c16[:], c16a[:])
            c32 = cpool.tile([128, 136], F32, name="c32")
            nc.scalar.dma_start(c32[:], cst32[:])
            ones32 = c16[0:128, 0:32]
            idt = c16[:, 32:160]
            bigmask = c32[:, 0:128]
            gamma128 = c32[:, 128:129]
            beta128 = c32[:, 129:130]
            epsb = c32[:, 130:131]
            biasC = c32[:, 131:132]

            s_sb = spool.tile([128, 16 * YT], F16, name="s_sb")
            sums = spool.tile([128, 8], F32, name="sums")
            sumsq = spool.tile([128, 8], F32, name="sumsq")

            xall = xpool.tile([128, NIMG * NPIX], F16, name="xall")
            nc.sync.dma_start(xall[:, 0 : 2 * NPIX], xh[:, 0 : 2 * NPIX])
            nc.sync.dma_start(
                xall[:, 2 * NPIX : 4 * NPIX], xh[:, 2 * NPIX : 4 * NPIX]
            )
            for b in range(1, NBLK):
                sl = slice(b * 4 * NPIX, (b + 1) * 4 * NPIX)
                nc.sync.dma_start(xall[:, sl], xh[:, sl])

            # squares on scalar ACT, issued with +2 lookahead so the in-order
            # scalar queue never makes a drain wait on a square
            uts = [
                upool.tile([128, 4 * NPIX], F16, name=f"ut{b}", tag=f"ut{b % 3}")
                for b in range(NBLK)
            ]

            def _issue_square_v(b):
                lo = b * 4 * NPIX
                nc.vector.tensor_mul(
                    uts[b][:, 0 : 2 * NPIX],
                    xall[:, lo : lo + 2 * NPIX],
                    xall[:, lo : lo + 2 * NPIX],
                )

            def _issue_square_s(b):
                lo = b * 4 * NPIX
                nc.scalar.activation(
                    uts[b][:, 2 * NPIX : 4 * NPIX],
                    xall[:, lo + 2 * NPIX : lo + 4 * NPIX],
                    mybir.ActivationFunctionType.Square,
                )

            def _issue_square(b):
                _issue_square_v(b)
                _issue_square_s(b)

            _issue_square(0)
            nc.scalar.activation(
                uts[1][:, 0 : 2 * NPIX],
                xall[:, 4 * NPIX : 6 * NPIX],
                mybir.ActivationFunctionType.Square,
            )
            _issue_square_s(1)
            _issue_square_s(2)

            def _issue_r4(b):
                ut = uts[b]
                r4 = psr.tile([128, 904], F32, name=f"r4_{b}", tag="r4")
                for j in (2, 3, 0, 1):
                    for lo, hi in ((0, 512), (512, 900)):
                        nc.tensor.matmul(
                            r4[32 * j : 32 * j + 32, lo:hi],
                            ones32,
                            ut[:, j * NPIX + lo : j * NPIX + hi],
                            start=True,
                            stop=True,
                            tile_position=(0, 32 * j),
                            skip_group_check=True,
                        )
                return r4

            r4s = {0: _issue_r4(0)}

            for b in range(NBLK):
                r4 = r4s.pop(b)
                rc16 = rpool.tile([128, 904], F16, name=f"rc16_{b}", tag="rc16")
                nc.vector.tensor_scalar(
                    rc16[:, 0:900],
                    r4[:, 0:900],
                    96.0,
                    None,
                    op0=mybir.AluOpType.subtract,
                )
                vv = rpool.tile([128, 840], F16, name=f"vv{b}", tag="vv")
                nc.vector.tensor_add(vv[:], rc16[:, 0:840], rc16[:, 30:870])
                nc.vector.tensor_add(vv[:], vv[:], rc16[:, 60:900])
                t1f = rpool.tile([128, 840], F16, name=f"t1f{b}", tag="t1f")
                nc.vector.tensor_add(t1f[:, 0:838], vv[:, 0:838], vv[:, 1:839])
                nc.vector.tensor_add(t1f[:, 0:838], t1f[:, 0:838], vv[:, 2:840])
                if b + 2 < NBLK:
                    _issue_square_v(b + 2)
                if b + 3 < NBLK:
                    _issue_square_s(b + 3)

                if b + 1 < NBLK:
                    r4s[b + 1] = _issue_r4(b + 1)
                x3 = xall[:].rearrange("p (n a b) -> p n a b", a=HP, b=HP)
                ps = psc.tile([128, 1024], F32, name=f"ps{b}", tag="ps")
                t13 = t1f[:].rearrange("p (a c) -> p a c", c=HP)
                for yt in range(2):
                    y0 = yt * YT_ROWS
                    nc.tensor.matmul(
                        ps[:, 512 * yt : 512 * yt + YT],
                        idt,
                        t13[:, y0 : y0 + YT_ROWS, 0:28],
                        start=True,
                        stop=False,
                        skip_group_check=True,
                    )
                for yt in range(2):
                    y0 = yt * YT_ROWS
                    for k in range(9):
                        dy, dx = divmod(k, 3)
                        for j in range(4):
                            nc.tensor.matmul(
                                ps[32 * j : 32 * j + 32, 512 * yt : 512 * yt + YT],
                                wtile[:, k * 32 : (k + 1) * 32],
                                x3[:, b * 4 + j, y0 + dy : y0 + dy + YT_ROWS, dx : dx + 28],
                                start=False,
                                stop=(k == 8),
                                tile_position=(0, 32 * j),
                                skip_group_check=True,
                            )
                blk = b * 2 * YT
                psv = AP(ps.tensor, ps.offset, [[1024, 128], [512, 2], [1, YT]])
                nc.scalar.activation(
                    s_sb[:, blk : blk + 2 * YT],
                    psv,
                    mybir.ActivationFunctionType.Copy,
                    bias=-288.0,
                    accum_out=sums[:, b : b + 1],
                )
                sq_scr = opool.tile([128, 2 * YT], F32, name="sq_scr", tag="sq")
                nc.scalar.activation(
                    sq_scr[:],
                    s_sb[:, blk : blk + 2 * YT],
                    mybir.ActivationFunctionType.Square,
                    accum_out=sumsq[:, b : b + 1],
                )

            # ---- stats tail: everything on [128, *] (channel stats
            # replicated across the 4 image slots by the bigmask matmul) ----
            sq2 = spool.tile([128, 2], F32, name="sq2")
            nc.vector.tensor_reduce(
                out=sq2[:, 0:1], in_=sums[:], op=mybir.AluOpType.add,
                axis=mybir.AxisListType.X,
            )
            nc.vector.tensor_reduce(
                out=sq2[:, 1:2], in_=sumsq[:], op=mybir.AluOpType.add,
                axis=mybir.AxisListType.X,
            )
            gstat = psr.tile([128, 2], F32, name="gstat", tag="r4")
            nc.tensor.matmul(gstat[:], bigmask, sq2[:], start=True, stop=True)
            ab = spool.tile([128, 8], F32, name="ab")
            nc.vector.tensor_copy(ab[:, 0:2], gstat[:])
            mean = ab[:, 0:1]
            qn = ab[:, 1:2]
            var = ab[:, 2:3]
            nc.vector.scalar_tensor_tensor(
                var, mean, 1.0, mean, op0=mybir.AluOpType.mult, op1=mybir.AluOpType.mult
            )
            nc.vector.tensor_sub(var, qn, var)
            abv = spool.tile([128, 2], F32, name="abv")
            A = abv[:, 0:1]
            B = abv[:, 1:2]
            sd = ab[:, 3:4]
            nc.scalar.activation(
                sd, var, mybir.ActivationFunctionType.Sqrt, bias=epsb
            )
            nc.vector.reciprocal(A, sd)
            nc.vector.tensor_mul(A, A, gamma128)
            nc.vector.scalar_tensor_tensor(
                B, mean, 1.0, A, op0=mybir.AluOpType.mult, op1=mybir.AluOpType.mult
            )
            nc.vector.tensor_sub(B, beta128, B)

            for b in range(NBLK):
                blk = b * 2 * YT
                o = opool.tile([128, 2 * YT], F16, name=f"o{b}", tag=f"o{b % 3}")
                if b % 3 == 0:
                    nc.vector.tensor_scalar(
                        o[:],
                        s_sb[:, blk : blk + 2 * YT],
                        A,
                        B,
                        op0=mybir.AluOpType.mult,
                        op1=mybir.AluOpType.add,
                    )
                elif b % 3 == 1:
                    nc.scalar.activation(
                        o[:],
                        s_sb[:, blk : blk + 2 * YT],
                        mybir.ActivationFunctionType.Identity,
                        bias=B,
                        scale=A,
                    )
                else:
                    nc.gpsimd.tensor_scalar(
                        o[:],
                        s_sb[:, blk : blk + 2 * YT],
                        A,
                        B,
                        op0=mybir.AluOpType.mult,
                        op1=mybir.AluOpType.add,
                    )
                dstap = AP(
                    y.ap().tensor,
                    b * 4 * 32 * NV,
                    [[32 * NV, 4], [NV, 32], [1, 2 * YT]],
                )
                eng = (nc.sync, nc.scalar, nc.gpsimd)[b % 3]
                eng.dma_start(dstap, o[:])
    return nc


def _prep_inputs(x, w, gamma, beta):
    x = np.asarray(x, np.float32)
    w = np.asarray(w, np.float32)
    gamma = np.asarray(gamma, np.float32)
    beta = np.asarray(beta, np.float32)

    xp = np.zeros((NIMG, 128, HP, HP), np.float32)
    xp[:, :, 1:29, 1:29] = x
    xh = np.ascontiguousarray(xp.transpose(1, 0, 2, 3)).reshape(128, NIMG * NPIX)
    xh = xh.astype(np.float16)

    c16a = np.zeros((128, 32 + 128), np.float16)
    c16a[:, 0:32] = 1.0
    c16a[:, 32:160] = np.eye(128, dtype=np.float16)

    bigmask = np.zeros((128, 128), np.float32)
    for c in range(128):
        bigmask[c, c % 32 :: 32] = 1.0
    # gstat128[p,:] = sum_q bigmask[q,p] * sq2[q,:]; want channel (p%32) sums
    # -> bigmask[q,p] = 1 iff q%32 == p%32
    bm = np.zeros((128, 128), np.float32)
    qq = np.arange(128)
    for p in range(128):
        bm[:, p] = (qq % 32 == p % 32).astype(np.float32) / NHW

    maps = []
    for core in range(N_CORES):
        wtc = (2.0 * w[core * 32 : (core + 1) * 32]).reshape(32, 128, 9)
        wtc = np.ascontiguousarray(wtc.transpose(1, 2, 0)).reshape(128, 9 * 32)
        cst32 = np.zeros((128, 136), np.float32)
        cst32[:, 0:128] = bm
        g4 = np.tile(gamma[core * 32 : (core + 1) * 32], 4)
        b4 = np.tile(beta[core * 32 : (core + 1) * 32], 4)
        cst32[:, 128] = g4
        cst32[:, 129] = b4
        cst32[:, 130] = EPS
        cst32[:, 131] = -288.0
        maps.append(
            {
                "xh": xh,
                "wt": wtc.astype(np.float16),
                "c16a": c16a,
                "cst32": cst32,
            }
        )
    return maps


_NC_CACHE = []


def kernel(x, w, gamma, beta):
    if not _NC_CACHE:
        _NC_CACHE.append(_build_nc())
    nc = _NC_CACHE[0]
    maps = _prep_inputs(x, w, gamma, beta)
    res = run_bass_kernel_spmd(nc, maps, core_ids=list(range(N_CORES)))
    out = np.concatenate([r["y"] for r in res.results], axis=1)
    return np.ascontiguousarray(out.astype(np.float32))
